# revision 1
# baseline (speedup 1.0000x reference)
"""CorrRatio (symmetric correlation-ratio loss) on 8 Trainium2 NeuronCores.

Strategy
--------
Input: y_true, y_pred f32 (1,1,128,128,128) -> N = 2^21 voxels, sharded
contiguously across 8 cores as [128, 2048] f32 tiles (all reductions are
order-independent, so contiguous sharding is exact).

Phase 1 (tiny NEFF): exact threshold-count ladders around the expected
0.01/0.99 quantile locations (+ coarse safety ladder + global min/max).
Host sums per-core counts (the "small all-reduce") and interpolates the
empirical CDF -> quantiles accurate to ~1e-4 absolute, which feeds the
output with ~1e-4 relative error (measured sensitivity is ~1:1).
Thresholds are runtime inputs, so a miss (non-randn data) only needs a
re-run with a refined ladder, not a recompile.

Phase 2 (main NEFF): normalize y~ = clip((tgt - f_min)/f_bin_size, 0, 32).
In normalized space the Parzen kernel is exp(-PT*(y~-(k+0.5))^2) with
PT = 2.355^2/2 a compile-time constant. Per (direction, bin):
ACT Square (bias=-(k+0.5)) -> ACT Exp (scale=-PT, accum_out = bin_counts
partial) -> DVE scalar_tensor_tensor w*x (accum_out = weighted-sum
partial). Per-partition partials land in columns of a stats tile; one PE
ones-matmul folds partitions; host does the final (B,C,nb) algebra in f64.
"""

from contextlib import ExitStack

import numpy as np

import concourse.bacc as bacc
import concourse.bass as bass
import concourse.mybir as mybir
import concourse.tile as tile
from concourse import bass_utils

F32 = mybir.dt.float32
BF16 = mybir.dt.bfloat16
ALU = mybir.AluOpType
ACTF = mybir.ActivationFunctionType

NB = 32
SR = 1.0 / 2.355
PT = 1.0 / (2.0 * SR * SR)          # normalized preterm = 2.355^2/2
EPS = float(np.finfo(np.float32).eps)
NCORES = 8
N = 128 * 128 * 128                  # 2097152 voxels
V = N // NCORES                      # 262144 per core
P = 128
F = V // P                           # 2048 free-dim per partition

# ---------------------------------------------------------------- ladders
Z99 = 2.3263478740408408             # N(0,1) 0.99 quantile (inputs are randn)
FINE_RUNGS = 16
FINE_DELTA = 0.005
COARSE = [-6.0 + 1.5 * j for j in range(9)]
NLAD = 2 * FINE_RUNGS + len(COARSE)  # 41 per tensor


def _default_ladder():
    lo = [-Z99 + FINE_DELTA * (j - FINE_RUNGS / 2 + 0.5) for j in range(FINE_RUNGS)]
    hi = [Z99 + FINE_DELTA * (j - FINE_RUNGS / 2 + 0.5) for j in range(FINE_RUNGS)]
    return lo + hi + COARSE


def _build_phase1():
    """Counts of (value >= t_j) for 2*NLAD runtime thresholds, plus min/max.

    cst layout [128, 4*NLAD]: cols [0,2N): thresholds (yt then yp),
    cols [2N,4N): negated thresholds (Sign biases)."""
    nc = bacc.Bacc("TRN2", target_bir_lowering=False, debug=False,
                   num_devices=NCORES)
    yt_d = nc.dram_tensor("yt", [P, F], F32, kind="ExternalInput").ap()
    yp_d = nc.dram_tensor("yp", [P, F], F32, kind="ExternalInput").ap()
    cst_d = nc.dram_tensor("cst", [P, 4 * NLAD], F32, kind="ExternalInput").ap()
    ncols = 2 * NLAD
    out_d = nc.dram_tensor("cnt", [1, ncols + 4], F32, kind="ExternalOutput").ap()

    with tile.TileContext(nc) as tc:
        with (
            tc.tile_pool(name="io", bufs=1) as io_pool,
            tc.tile_pool(name="scr", bufs=4) as scr_pool,
            tc.tile_pool(name="stat", bufs=1) as stat_pool,
            tc.tile_pool(name="psum", bufs=1, space="PSUM") as psum_pool,
        ):
            yt = io_pool.tile([P, F], F32)
            yp = io_pool.tile([P, F], F32)
            cst = io_pool.tile([P, 4 * NLAD], F32)
            nc.sync.dma_start(cst[:], cst_d)
            nc.sync.dma_start(yt[:], yt_d)
            nc.sync.dma_start(yp[:], yp_d)

            cnt = stat_pool.tile([P, ncols], F32)      # per-partition counts
            mm = stat_pool.tile([P, 4], F32)           # min/max per partition
            ones = stat_pool.tile([P, 1], F32)
            nc.vector.memset(ones[:], 1.0)

            # Alternate DVE(is_ge) / ACT(Sign): DVE op ~1.13us, ACT ~1.89us,
            # so give DVE 5 of every 8 ops.
            for col in range(ncols):
                src = yt if col < NLAD else yp
                if col % 8 < 5:
                    scr = scr_pool.tile([P, F], BF16, tag="scr")
                    nc.vector.tensor_scalar(
                        out=scr[:], in0=src[:], scalar1=cst[:, col:col + 1],
                        scalar2=None, op0=ALU.is_ge, op1=ALU.add,
                        accum_out=cnt[:, col:col + 1])
                else:
                    scr = scr_pool.tile([P, F], BF16, tag="ascr")
                    nc.scalar.activation(
                        scr[:], src[:], ACTF.Sign,
                        bias=cst[:, ncols + col:ncols + col + 1],
                        scale=1.0, accum_out=cnt[:, col:col + 1])

            # cols 0/2 hold NEGATED mins (cross-lane reduce only does max).
            nc.vector.tensor_reduce(mm[:, 0:1], yt[:], axis=mybir.AxisListType.X, op=ALU.min)
            nc.vector.tensor_reduce(mm[:, 1:2], yt[:], axis=mybir.AxisListType.X, op=ALU.max)
            nc.vector.tensor_reduce(mm[:, 2:3], yp[:], axis=mybir.AxisListType.X, op=ALU.min)
            nc.vector.tensor_reduce(mm[:, 3:4], yp[:], axis=mybir.AxisListType.X, op=ALU.max)
            nc.vector.tensor_scalar(out=mm[:, 0:1], in0=mm[:, 0:1], scalar1=-1.0,
                                    scalar2=None, op0=ALU.mult)
            nc.vector.tensor_scalar(out=mm[:, 2:3], in0=mm[:, 2:3], scalar1=-1.0,
                                    scalar2=None, op0=ALU.mult)

            # Fold partitions: counts via PE ones-matmul; min/max via GPSIMD.
            psum = psum_pool.tile([1, ncols], F32)
            nc.tensor.matmul(psum[:], ones[:], cnt[:], start=True, stop=True)
            osb = stat_pool.tile([1, ncols + 4], F32)
            nc.scalar.copy(osb[:, 0:ncols], psum[:])
            for j in range(4):
                nc.gpsimd.tensor_reduce(osb[:, ncols + j:ncols + j + 1],
                                        mm[:, j:j + 1],
                                        axis=mybir.AxisListType.C, op=ALU.max)
            nc.sync.dma_start(out_d, osb[:])
    nc.compile()
    return nc


def _build_phase2():
    """Main pass. Direction A: target=y_pred, pred=y_true (mirrors
    correlation_ratio(y_true, y_pred)); direction B swaps.

    cst layout [128, 40]: cols [0,32): -(k+0.5); col 32+4*di+{0..3}:
    (f_min, inv_fbs, m_min, m_max) for direction di."""
    nc = bacc.Bacc("TRN2", target_bir_lowering=False, debug=False,
                   num_devices=NCORES)
    yt_d = nc.dram_tensor("yt", [P, F], F32, kind="ExternalInput").ap()
    yp_d = nc.dram_tensor("yp", [P, F], F32, kind="ExternalInput").ap()
    cst_d = nc.dram_tensor("cst", [P, 40], F32, kind="ExternalInput").ap()
    # per direction: 32 S0 | 32 S1 | sumx | sumx2 -> 66 cols; A then B
    out_d = nc.dram_tensor("stats", [1, 132], F32, kind="ExternalOutput").ap()

    with tile.TileContext(nc) as tc:
        with (
            tc.tile_pool(name="io", bufs=1) as io_pool,
            tc.tile_pool(name="norm", bufs=1) as norm_pool,
            tc.tile_pool(name="scr", bufs=3) as scr_pool,
            tc.tile_pool(name="stat", bufs=1) as stat_pool,
            tc.tile_pool(name="psum", bufs=1, space="PSUM") as psum_pool,
        ):
            yt = io_pool.tile([P, F], F32)
            yp = io_pool.tile([P, F], F32)
            cst = io_pool.tile([P, 40], F32)
            nc.sync.dma_start(cst[:], cst_d)
            nc.sync.dma_start(yt[:], yt_d)
            nc.sync.dma_start(yp[:], yp_d)

            stats = stat_pool.tile([P, 132], F32)

            for di in range(2):
                tgt = yp if di == 0 else yt
                prd = yt if di == 0 else yp
                base = 66 * di
                cb = 32 + 4 * di
                f_min = cst[:, cb + 0:cb + 1]
                inv_fbs = cst[:, cb + 1:cb + 2]
                m_min = cst[:, cb + 2:cb + 3]
                m_max = cst[:, cb + 3:cb + 4]

                yn = norm_pool.tile([P, F], F32, tag=f"yn{di}")
                xc = norm_pool.tile([P, F], F32, tag=f"xc{di}")
                # y~ = clip((tgt - f_min)*inv_fbs, 0, 32)
                yraw = scr_pool.tile([P, F], F32, tag="sq")
                nc.vector.tensor_scalar(
                    out=yraw[:], in0=tgt[:], scalar1=f_min,
                    scalar2=inv_fbs, op0=ALU.subtract, op1=ALU.mult)
                nc.vector.tensor_scalar(
                    out=yn[:], in0=yraw[:], scalar1=float(NB),
                    scalar2=0.0, op0=ALU.min, op1=ALU.max)
                # x~ = clip(prd, m_min, m_max)
                nc.vector.tensor_scalar(
                    out=xc[:], in0=prd[:], scalar1=m_max,
                    scalar2=m_min, op0=ALU.min, op1=ALU.max)
                nc.vector.tensor_reduce(stats[:, base + 64:base + 65], xc[:],
                                        axis=mybir.AxisListType.X, op=ALU.add)
                # sum(x^2)
                xsq = scr_pool.tile([P, F], F32, tag="w")
                nc.vector.scalar_tensor_tensor(
                    out=xsq[:], in0=xc[:], scalar=1.0, in1=xc[:],
                    op0=ALU.mult, op1=ALU.mult,
                    accum_out=stats[:, base + 65:base + 66])

                for k in range(NB):
                    sq = scr_pool.tile([P, F], F32, tag="sq")
                    nc.scalar.activation(sq[:], yn[:], ACTF.Square,
                                         bias=cst[:, k:k + 1], scale=1.0)
                    w = scr_pool.tile([P, F], F32, tag="w")
                    nc.scalar.activation(w[:], sq[:], ACTF.Exp,
                                         bias=0.0, scale=-PT,
                                         accum_out=stats[:, base + k:base + k + 1])
                    wx = scr_pool.tile([P, F], F32, tag="wx")
                    nc.vector.scalar_tensor_tensor(
                        out=wx[:], in0=w[:], scalar=1.0, in1=xc[:],
                        op0=ALU.mult, op1=ALU.mult,
                        accum_out=stats[:, base + 32 + k:base + 32 + k + 1])

            ones = stat_pool.tile([P, 1], F32)
            nc.vector.memset(ones[:], 1.0)
            psum = psum_pool.tile([1, 132], F32)
            nc.tensor.matmul(psum[:], ones[:], stats[:], start=True, stop=True)
            osb = stat_pool.tile([1, 132], F32)
            nc.scalar.copy(osb[:], psum[:])
            nc.sync.dma_start(out_d, osb[:])
    nc.compile()
    return nc


_NC_CACHE = {}


def _get_nc(which):
    if which not in _NC_CACHE:
        _NC_CACHE[which] = _build_phase1() if which == "p1" else _build_phase2()
    return _NC_CACHE[which]


def _run(nc, in_maps, trace=False):
    return bass_utils.run_bass_kernel_spmd(
        nc, in_maps, core_ids=list(range(NCORES)), trace=trace)


def _p1_cst(ladder_yt, ladder_yp):
    thr = np.array(list(ladder_yt) + list(ladder_yp), dtype=np.float32)
    cst = np.concatenate([thr, -thr]).reshape(1, -1)
    return np.ascontiguousarray(np.broadcast_to(cst, (P, 4 * NLAD)), dtype=np.float32)


def _p2_cst(qyt_lo, qyt_hi, qyp_lo, qyp_hi):
    row = np.zeros(40, dtype=np.float32)
    row[:NB] = -(np.arange(NB, dtype=np.float32) + 0.5)
    for di, ((tlo, thi), (plo, phi)) in enumerate(
        (((qyp_lo, qyp_hi), (qyt_lo, qyt_hi)),
         ((qyt_lo, qyt_hi), (qyp_lo, qyp_hi)))):
        tlo32 = np.float32(tlo); thi32 = np.float32(thi)
        fbs = np.float32((thi32 - tlo32) / NB)
        row[32 + 4 * di + 0] = tlo32
        row[32 + 4 * di + 1] = np.float32(1.0) / fbs
        row[32 + 4 * di + 2] = np.float32(plo)
        row[32 + 4 * di + 3] = np.float32(phi)
    return np.ascontiguousarray(np.broadcast_to(row.reshape(1, -1), (P, 40)),
                                dtype=np.float32)


def _interp_quantile(thresholds, counts_ge, pos):
    """CDF interpolation: counts_ge[i] = #(values >= t_i) globally.
    pos = q*(N-1) fractional order-statistic position (ascending)."""
    below = N - np.asarray(counts_ge, dtype=np.float64)   # count(< t_i)
    r = pos + 1.0
    best = None
    for i in range(len(thresholds) - 1):
        if thresholds[i + 1] <= thresholds[i]:
            continue
        if below[i] <= r <= below[i + 1] and below[i + 1] > below[i]:
            frac = (r - below[i]) / (below[i + 1] - below[i])
            est = thresholds[i] + frac * (thresholds[i + 1] - thresholds[i])
            width = thresholds[i + 1] - thresholds[i]
            if best is None or width < best[0]:
                best = (width, est)
    return None if best is None else best[1]


def _quantiles_from_counts(ladder, counts_ge):
    nf = FINE_RUNGS
    q01 = _interp_quantile(ladder[:nf], counts_ge[:nf], 0.01 * (N - 1))
    q99 = _interp_quantile(ladder[nf:2 * nf], counts_ge[nf:2 * nf], 0.99 * (N - 1))
    co_thr, co_cnt = ladder[2 * nf:], counts_ge[2 * nf:]
    ok = (q01 is not None, q99 is not None)
    if q01 is None:
        q01 = _interp_quantile(co_thr, co_cnt, 0.01 * (N - 1))
    if q99 is None:
        q99 = _interp_quantile(co_thr, co_cnt, 0.99 * (N - 1))
    return (q01, ok[0]), (q99, ok[1])


def _counts_from_phase1(res_cnt):
    arr = np.stack([np.asarray(r, dtype=np.float64).reshape(-1) for r in res_cnt])
    tot = arr[:, :2 * NLAD].sum(axis=0)
    counts = np.empty(2 * NLAD)
    for col in range(2 * NLAD):
        if col % 8 < 5:
            counts[col] = tot[col]                      # is_ge count
        else:
            counts[col] = 0.5 * (tot[col] + N)          # sign sum -> count>=
    mm = arr[:, 2 * NLAD:]
    return counts, (-mm[:, 0].max(), mm[:, 1].max(), -mm[:, 2].max(), mm[:, 3].max())


def _final_algebra(stats_sum):
    out = 0.0
    for di in range(2):
        base = 66 * di
        S0 = stats_sum[base:base + 32]
        S1 = stats_sum[base + 32:base + 64]
        SX = stats_sum[base + 64]
        SX2 = stats_sum[base + 65]
        tm = SX / N
        mi = S1 / (S0 + EPS)
        bgv = float((S0 * (mi - tm) ** 2).sum() / (S0.sum() + EPS))
        tv = (SX2 - N * tm * tm) / (N - 1)
        out += bgv / (tv + EPS)
    return -out / 2.0


def kernel(y_true, y_pred):
    yt = np.ascontiguousarray(np.asarray(y_true, dtype=np.float32).reshape(-1))
    yp = np.ascontiguousarray(np.asarray(y_pred, dtype=np.float32).reshape(-1))
    assert yt.size == N and yp.size == N
    yt_sh = yt.reshape(NCORES, P, F)
    yp_sh = yp.reshape(NCORES, P, F)

    # ---- phase 1: quantiles
    ladder = _default_ladder()
    ladder_yt = ladder_yp = ladder
    nc1 = _get_nc("p1")
    for _attempt in range(3):
        cst1 = _p1_cst(ladder_yt, ladder_yp)
        in_maps = [{"yt": yt_sh[c], "yp": yp_sh[c], "cst": cst1}
                   for c in range(NCORES)]
        r1 = _run(nc1, in_maps)
        counts, mm = _counts_from_phase1([r["cnt"] for r in r1.results])
        (qyt_lo, ok1), (qyt_hi, ok2) = _quantiles_from_counts(ladder_yt, counts[:NLAD])
        (qyp_lo, ok3), (qyp_hi, ok4) = _quantiles_from_counts(ladder_yp, counts[NLAD:])
        if ok1 and ok2 and ok3 and ok4:
            break
        # Fine ladders missed (data not ~randn): refine around the coarse
        # estimates using the same compiled NEFF.
        gmin = min(mm[0], mm[2]) - 1e-3
        gmax = max(mm[1], mm[3]) + 1e-3
        delta = max((gmax - gmin) / 4096.0, 1e-7)
        coarse = list(np.linspace(gmin, gmax, len(COARSE)))

        def lad(lo_c, hi_c):
            return ([lo_c + delta * (j - FINE_RUNGS / 2 + 0.5) for j in range(FINE_RUNGS)]
                    + [hi_c + delta * (j - FINE_RUNGS / 2 + 0.5) for j in range(FINE_RUNGS)]
                    + coarse)
        ladder_yt = lad(qyt_lo, qyt_hi)
        ladder_yp = lad(qyp_lo, qyp_hi)

    # ---- phase 2: main pass
    cst2 = _p2_cst(qyt_lo, qyt_hi, qyp_lo, qyp_hi)
    nc2 = _get_nc("p2")
    in_maps2 = [{"yt": yt_sh[c], "yp": yp_sh[c], "cst": cst2}
                for c in range(NCORES)]
    r2 = _run(nc2, in_maps2)
    stats = np.stack([np.asarray(r["stats"], dtype=np.float64).reshape(-1)
                      for r in r2.results]).sum(axis=0)
    return np.array(_final_algebra(stats), dtype=np.float32)



# revision 2
# speedup vs baseline: 1.9850x; 1.9850x over previous
"""CorrRatio (symmetric correlation-ratio loss) on 8 Trainium2 NeuronCores.

Strategy
--------
Input: y_true, y_pred f32 (1,1,128,128,128) -> N = 2^21 voxels, sharded
contiguously across 8 cores as [128, 2048] f32 tiles (all reductions are
order-independent, so contiguous sharding is exact).

Phase 1 (slim NEFF): exact threshold-count ladders around the expected
0.01/0.99 quantile locations (randn inputs) + 2 coarse safety rungs per
tensor. Counts split between ACT (Sign+accum) and DVE (is_ge+accum) so
both engines run in parallel. Host sums per-core counts (the "small
all-reduce") and interpolates the empirical CDF -> quantiles. Thresholds
are runtime inputs; a miss (non-randn data) re-runs the same NEFF with a
refined ladder, no recompile.

Phase 2 (main NEFF): yn = clip((tgt - f_min)*inv_fbs, 0, 32). Per bin:
ONE ACT op  Derivative_Erf(sqrt(PT)*yn - sqrt(PT)*(k+.5))
          = (2/sqrt(pi)) * exp(-PT*(yn-c_k)^2)   [bf16 out]
with accum_out -> S0[k] partials, and ONE DVE scalar_tensor_tensor
w*x (bf16) with accum_out -> S1[k] partials. Key measured facts: ACT is
~2.0us/2048 elems dtype-independent; ANY accumulating DVE op is 1x mode
(~2.2us) — so 1 ACT + 1 DVE op per bin saturates both engines evenly.
Per-partition partials fold via one PE ones-matmul; host does the final
(B,C,nb) algebra in f64 (the 2/sqrt(pi) scale cancels in the ratios).
"""

import numpy as np

import concourse.bacc as bacc
import concourse.bass as bass
import concourse.mybir as mybir
import concourse.tile as tile
from concourse import bass_utils

F32 = mybir.dt.float32
BF16 = mybir.dt.bfloat16
ALU = mybir.AluOpType
ACTF = mybir.ActivationFunctionType

NB = 32
SR = 1.0 / 2.355
PT = 1.0 / (2.0 * SR * SR)          # normalized preterm = 2.355^2/2
SQPT = float(np.sqrt(PT))
EPS = float(np.finfo(np.float32).eps)
NCORES = 8
N = 128 * 128 * 128                  # 2097152 voxels
V = N // NCORES                      # 262144 per core
P = 128
F = V // P                           # 2048 free-dim per partition

# ---------------------------------------------------------------- ladders
Z99 = 2.3263478740408408             # N(0,1) 0.99 quantile (inputs are randn)
FINE_RUNGS = 4                       # per quantile
FINE_DELTA = 0.005
COARSE = [-4.0, -2.0, 2.0, 4.0]      # fallback bracket rungs
NLAD = 2 * FINE_RUNGS + len(COARSE)  # 12 per tensor


def _default_ladder():
    lo = [-Z99 + FINE_DELTA * (j - FINE_RUNGS / 2 + 0.5) for j in range(FINE_RUNGS)]
    hi = [Z99 + FINE_DELTA * (j - FINE_RUNGS / 2 + 0.5) for j in range(FINE_RUNGS)]
    return lo + hi + COARSE


def _build_phase1():
    """Counts of (value >= t_j) for 2*NLAD runtime thresholds.

    cst layout [128, 4*NLAD]: cols [0,2N): thresholds (yt then yp),
    cols [2N,4N): negated thresholds (Sign biases). Rungs alternate
    between DVE (is_ge+accum, 2.2us) and ACT (Sign+accum, 2.2us)."""
    nc = bacc.Bacc("TRN2", target_bir_lowering=False, debug=False,
                   num_devices=NCORES)
    yt_d = nc.dram_tensor("yt", [P, F], F32, kind="ExternalInput").ap()
    yp_d = nc.dram_tensor("yp", [P, F], F32, kind="ExternalInput").ap()
    cst_d = nc.dram_tensor("cst", [P, 4 * NLAD], F32, kind="ExternalInput").ap()
    ncols = 2 * NLAD
    out_d = nc.dram_tensor("cnt", [1, ncols], F32, kind="ExternalOutput").ap()

    with tile.TileContext(nc) as tc:
        with (
            tc.tile_pool(name="io", bufs=1) as io_pool,
            tc.tile_pool(name="scr", bufs=4) as scr_pool,
            tc.tile_pool(name="stat", bufs=1) as stat_pool,
            tc.tile_pool(name="psum", bufs=1, space="PSUM") as psum_pool,
        ):
            yt = io_pool.tile([P, F], F32)
            yp = io_pool.tile([P, F], F32)
            cst = io_pool.tile([P, 4 * NLAD], F32)
            nc.sync.dma_start(cst[:], cst_d)
            nc.sync.dma_start(yt[:], yt_d)
            nc.sync.dma_start(yp[:], yp_d)

            cnt = stat_pool.tile([P, ncols], F32)      # per-partition counts
            ones = stat_pool.tile([P, 1], F32)
            nc.vector.memset(ones[:], 1.0)

            # Even cols -> DVE is_ge, odd cols -> ACT Sign: both ~2.2us.
            for col in range(ncols):
                src = yt if col < NLAD else yp
                if col % 2 == 0:
                    scr = scr_pool.tile([P, F], BF16, tag="scr")
                    nc.vector.tensor_scalar(
                        out=scr[:], in0=src[:], scalar1=cst[:, col:col + 1],
                        scalar2=None, op0=ALU.is_ge, op1=ALU.add,
                        accum_out=cnt[:, col:col + 1])
                else:
                    scr = scr_pool.tile([P, F], BF16, tag="ascr")
                    nc.scalar.activation(
                        scr[:], src[:], ACTF.Sign,
                        bias=cst[:, ncols + col:ncols + col + 1],
                        scale=1.0, accum_out=cnt[:, col:col + 1])

            # Fold partitions via PE ones-matmul.
            psum = psum_pool.tile([1, ncols], F32)
            nc.tensor.matmul(psum[:], ones[:], cnt[:], start=True, stop=True)
            osb = stat_pool.tile([1, ncols], F32)
            nc.scalar.copy(osb[:], psum[:])
            nc.sync.dma_start(out_d, osb[:])
    nc.compile()
    return nc


def _build_phase2():
    """Main pass. Direction A: target=y_pred, pred=y_true (mirrors
    correlation_ratio(y_true, y_pred)); direction B swaps.

    cst layout [128, 40]: cols [0,32): -sqrt(PT)*(k+0.5); col 32+4*di+{0..3}:
    (f_min, inv_fbs, m_min, m_max) for direction di."""
    nc = bacc.Bacc("TRN2", target_bir_lowering=False, debug=False,
                   num_devices=NCORES)
    yt_d = nc.dram_tensor("yt", [P, F], F32, kind="ExternalInput").ap()
    yp_d = nc.dram_tensor("yp", [P, F], F32, kind="ExternalInput").ap()
    cst_d = nc.dram_tensor("cst", [P, 40], F32, kind="ExternalInput").ap()
    # per direction: 32 S0 | 32 S1 | sumx | sumx2 -> 66 cols; A then B
    out_d = nc.dram_tensor("stats", [1, 132], F32, kind="ExternalOutput").ap()

    with tile.TileContext(nc) as tc:
        with (
            tc.tile_pool(name="io", bufs=1) as io_pool,
            tc.tile_pool(name="norm", bufs=1) as norm_pool,
            tc.tile_pool(name="w", bufs=4) as w_pool,
            tc.tile_pool(name="scr", bufs=3) as scr_pool,
            tc.tile_pool(name="stat", bufs=1) as stat_pool,
            tc.tile_pool(name="psum", bufs=1, space="PSUM") as psum_pool,
        ):
            yt = io_pool.tile([P, F], F32)
            yp = io_pool.tile([P, F], F32)
            cst = io_pool.tile([P, 40], F32)
            nc.sync.dma_start(cst[:], cst_d)
            nc.sync.dma_start(yt[:], yt_d)
            nc.sync.dma_start(yp[:], yp_d)

            stats = stat_pool.tile([P, 132], F32)

            for di in range(2):
                tgt = yp if di == 0 else yt
                prd = yt if di == 0 else yp
                base = 66 * di
                cb = 32 + 4 * di
                f_min = cst[:, cb + 0:cb + 1]
                inv_fbs = cst[:, cb + 1:cb + 2]
                m_min = cst[:, cb + 2:cb + 3]
                m_max = cst[:, cb + 3:cb + 4]

                yn = norm_pool.tile([P, F], F32, tag=f"yn{di}")
                xb = norm_pool.tile([P, F], BF16, tag=f"xb{di}")
                # y~ = clip((tgt - f_min)*inv_fbs, 0, 32)   [2 DVE ts, no accum]
                yraw = scr_pool.tile([P, F], F32, tag="yraw")
                nc.vector.tensor_scalar(
                    out=yraw[:], in0=tgt[:], scalar1=f_min,
                    scalar2=inv_fbs, op0=ALU.subtract, op1=ALU.mult)
                nc.vector.tensor_scalar(
                    out=yn[:], in0=yraw[:], scalar1=float(NB),
                    scalar2=0.0, op0=ALU.min, op1=ALU.max)
                # x~ = clip(prd, m_min, m_max), bf16 [DVE ts, no accum]
                nc.vector.tensor_scalar(
                    out=xb[:], in0=prd[:], scalar1=m_max,
                    scalar2=m_min, op0=ALU.min, op1=ALU.max)
                # SX = sum(x~) via ACT Copy+accum (keeps DVE free)
                xcp = scr_pool.tile([P, F], BF16, tag="xcp")
                nc.scalar.activation(xcp[:], xb[:], ACTF.Copy,
                                     bias=0.0, scale=1.0,
                                     accum_out=stats[:, base + 64:base + 65])
                # SX2 = sum(x~^2): TT square (bf16 2x) + ACT Copy+accum
                xsq = scr_pool.tile([P, F], BF16, tag="xsq")
                nc.vector.tensor_tensor(out=xsq[:], in0=xb[:], in1=xb[:],
                                        op=ALU.mult)
                xsqc = scr_pool.tile([P, F], BF16, tag="xsqc")
                nc.scalar.activation(xsqc[:], xsq[:], ACTF.Copy,
                                     bias=0.0, scale=1.0,
                                     accum_out=stats[:, base + 65:base + 66])

                for k in range(NB):
                    # w = (2/sqrt(pi)) exp(-PT (yn - k - .5)^2)   [1 ACT op]
                    w = w_pool.tile([P, F], BF16, tag="w")
                    nc.scalar.activation(w[:], yn[:], ACTF.Derivative_Erf,
                                         bias=cst[:, k:k + 1], scale=SQPT,
                                         accum_out=stats[:, base + k:base + k + 1])
                    # S1[k] += sum(w * x~)   [1 DVE op]
                    wx = scr_pool.tile([P, F], BF16, tag="wx")
                    nc.vector.scalar_tensor_tensor(
                        out=wx[:], in0=w[:], scalar=1.0, in1=xb[:],
                        op0=ALU.mult, op1=ALU.mult,
                        accum_out=stats[:, base + 32 + k:base + 32 + k + 1])

            ones = stat_pool.tile([P, 1], F32)
            nc.vector.memset(ones[:], 1.0)
            psum = psum_pool.tile([1, 132], F32)
            nc.tensor.matmul(psum[:], ones[:], stats[:], start=True, stop=True)
            osb = stat_pool.tile([1, 132], F32)
            nc.scalar.copy(osb[:], psum[:])
            nc.sync.dma_start(out_d, osb[:])
    nc.compile()
    return nc


_NC_CACHE = {}


def _get_nc(which):
    if which not in _NC_CACHE:
        _NC_CACHE[which] = _build_phase1() if which == "p1" else _build_phase2()
    return _NC_CACHE[which]


def _run(nc, in_maps, trace=False):
    return bass_utils.run_bass_kernel_spmd(
        nc, in_maps, core_ids=list(range(NCORES)), trace=trace)


def _p1_cst(ladder_yt, ladder_yp):
    thr = np.array(list(ladder_yt) + list(ladder_yp), dtype=np.float32)
    cst = np.concatenate([thr, -thr]).reshape(1, -1)
    return np.ascontiguousarray(np.broadcast_to(cst, (P, 4 * NLAD)), dtype=np.float32)


def _p2_cst(qyt_lo, qyt_hi, qyp_lo, qyp_hi):
    row = np.zeros(40, dtype=np.float32)
    row[:NB] = -SQPT * (np.arange(NB, dtype=np.float32) + 0.5)
    for di, ((tlo, thi), (plo, phi)) in enumerate(
        (((qyp_lo, qyp_hi), (qyt_lo, qyt_hi)),
         ((qyt_lo, qyt_hi), (qyp_lo, qyp_hi)))):
        tlo32 = np.float32(tlo); thi32 = np.float32(thi)
        fbs = np.float32((thi32 - tlo32) / NB)
        row[32 + 4 * di + 0] = tlo32
        row[32 + 4 * di + 1] = np.float32(1.0) / fbs
        row[32 + 4 * di + 2] = np.float32(plo)
        row[32 + 4 * di + 3] = np.float32(phi)
    return np.ascontiguousarray(np.broadcast_to(row.reshape(1, -1), (P, 40)),
                                dtype=np.float32)


def _interp_quantile(thresholds, counts_ge, pos):
    """CDF interpolation: counts_ge[i] = #(values >= t_i) globally.
    pos = q*(N-1) fractional order-statistic position (ascending)."""
    below = N - np.asarray(counts_ge, dtype=np.float64)   # count(< t_i)
    r = pos + 1.0
    best = None
    for i in range(len(thresholds) - 1):
        if thresholds[i + 1] <= thresholds[i]:
            continue
        if below[i] <= r <= below[i + 1] and below[i + 1] > below[i]:
            frac = (r - below[i]) / (below[i + 1] - below[i])
            est = thresholds[i] + frac * (thresholds[i + 1] - thresholds[i])
            width = thresholds[i + 1] - thresholds[i]
            if best is None or width < best[0]:
                best = (width, est)
    return None if best is None else best[1]


def _quantiles_from_counts(ladder, counts_ge):
    nf = FINE_RUNGS
    q01 = _interp_quantile(ladder[:nf], counts_ge[:nf], 0.01 * (N - 1))
    q99 = _interp_quantile(ladder[nf:2 * nf], counts_ge[nf:2 * nf], 0.99 * (N - 1))
    co_thr, co_cnt = ladder[2 * nf:], counts_ge[2 * nf:]
    ok = (q01 is not None, q99 is not None)
    if q01 is None:
        q01 = _interp_quantile(co_thr, co_cnt, 0.01 * (N - 1))
    if q99 is None:
        q99 = _interp_quantile(co_thr, co_cnt, 0.99 * (N - 1))
    return (q01, ok[0]), (q99, ok[1])


def _counts_from_phase1(res_cnt):
    arr = np.stack([np.asarray(r, dtype=np.float64).reshape(-1) for r in res_cnt])
    tot = arr.sum(axis=0)
    counts = np.empty(2 * NLAD)
    for col in range(2 * NLAD):
        if col % 2 == 0:
            counts[col] = tot[col]                      # is_ge count
        else:
            counts[col] = 0.5 * (tot[col] + N)          # sign sum -> count>=
    return counts


def _final_algebra(stats_sum):
    out = 0.0
    for di in range(2):
        base = 66 * di
        S0 = stats_sum[base:base + 32]
        S1 = stats_sum[base + 32:base + 64]
        SX = stats_sum[base + 64]
        SX2 = stats_sum[base + 65]
        # S0/S1 carry the DErf 2/sqrt(pi) factor; it cancels in mi and bgv.
        tm = SX / N
        mi = S1 / (S0 + EPS)
        bgv = float((S0 * (mi - tm) ** 2).sum() / (S0.sum() + EPS))
        tv = (SX2 - N * tm * tm) / (N - 1)
        out += bgv / (tv + EPS)
    return -out / 2.0


def kernel(y_true, y_pred):
    yt = np.ascontiguousarray(np.asarray(y_true, dtype=np.float32).reshape(-1))
    yp = np.ascontiguousarray(np.asarray(y_pred, dtype=np.float32).reshape(-1))
    assert yt.size == N and yp.size == N
    yt_sh = yt.reshape(NCORES, P, F)
    yp_sh = yp.reshape(NCORES, P, F)

    # ---- phase 1: quantiles
    ladder = _default_ladder()
    ladder_yt = ladder_yp = ladder
    nc1 = _get_nc("p1")
    for _attempt in range(4):
        cst1 = _p1_cst(ladder_yt, ladder_yp)
        in_maps = [{"yt": yt_sh[c], "yp": yp_sh[c], "cst": cst1}
                   for c in range(NCORES)]
        r1 = _run(nc1, in_maps)
        counts = _counts_from_phase1([r["cnt"] for r in r1.results])
        (qyt_lo, ok1), (qyt_hi, ok2) = _quantiles_from_counts(ladder_yt, counts[:NLAD])
        (qyp_lo, ok3), (qyp_hi, ok4) = _quantiles_from_counts(ladder_yp, counts[NLAD:])
        if ok1 and ok2 and ok3 and ok4:
            break
        # Fine ladders missed (data not ~randn): refine around the coarse
        # bracket using the same compiled NEFF. If even the coarse bracket
        # missed (quantile is None), widen it geometrically.
        span = max(abs(c) for c in COARSE)
        wide = [-span * 8, -span * 4, span * 4, span * 8]

        def lad(lo_c, hi_c, lo_w, hi_w):
            if lo_c is None or hi_c is None:
                return ([lo_w + (hi_w - lo_w) * j / (2 * FINE_RUNGS - 1)
                         for j in range(2 * FINE_RUNGS)] + wide)
            d = max((hi_c - lo_c) / 256.0, 1e-7)
            return ([lo_c + d * (j - FINE_RUNGS / 2 + 0.5) for j in range(FINE_RUNGS)]
                    + [hi_c + d * (j - FINE_RUNGS / 2 + 0.5) for j in range(FINE_RUNGS)]
                    + [lo_c - 32 * d, lo_c + 32 * d, hi_c - 32 * d, hi_c + 32 * d])
        ladder_yt = lad(qyt_lo, qyt_hi, -span * 2, span * 2)
        ladder_yp = lad(qyp_lo, qyp_hi, -span * 2, span * 2)

    # ---- phase 2: main pass
    cst2 = _p2_cst(qyt_lo, qyt_hi, qyp_lo, qyp_hi)
    nc2 = _get_nc("p2")
    in_maps2 = [{"yt": yt_sh[c], "yp": yp_sh[c], "cst": cst2}
                for c in range(NCORES)]
    r2 = _run(nc2, in_maps2)
    stats = np.stack([np.asarray(r["stats"], dtype=np.float64).reshape(-1)
                      for r in r2.results]).sum(axis=0)
    return np.array(_final_algebra(stats), dtype=np.float32)


# revision 3
# speedup vs baseline: 2.0701x; 1.0429x over previous
"""CorrRatio (symmetric correlation-ratio loss) on 8 Trainium2 NeuronCores.

Strategy
--------
Input: y_true, y_pred f32 (1,1,128,128,128) -> N = 2^21 voxels, sharded
contiguously across 8 cores as [128, 2048] f32 tiles (all reductions are
order-independent, so contiguous sharding is exact).

Phase 1 (slim NEFF): exact threshold-count ladders around the expected
0.01/0.99 quantile locations (randn inputs) + 2 coarse safety rungs per
tensor. Counts split between ACT (Sign+accum) and DVE (is_ge+accum) so
both engines run in parallel; per-partition counts are folded on host
(the "small all-reduce"). Thresholds are runtime inputs; a miss
(non-randn data) re-runs the same NEFF with a refined ladder.

Phase 2 (main NEFF): per tensor one DVE clip op (f32) + one clipped bf16
cast. Per (direction, bin): ONE ACT op
  Derivative_Erf(s*tc + b_k) = (2/sqrt(pi)) * exp(-PT*(yn - k - .5)^2)
(the normalize affine yn=(tc-f_min)*inv_fbs is folded into scale/bias;
bf16 out) with accum_out -> S0[k], and ONE DVE scalar_tensor_tensor
w*x (bf16) with accum_out -> S1[k]. Measured facts driving this: ACT is
~2.0us/2048 elems dtype-independent; ANY accumulating DVE op is 1x mode
(~2.2us) — so 1 ACT + 1 DVE op per bin saturates both engines evenly.
Host folds partition partials and does the final algebra in f64 (the
DErf 2/sqrt(pi) scale cancels in the ratios).
"""

import numpy as np

import concourse.bacc as bacc
import concourse.bass as bass
import concourse.mybir as mybir
import concourse.tile as tile
from concourse import bass_utils

F32 = mybir.dt.float32
BF16 = mybir.dt.bfloat16
ALU = mybir.AluOpType
ACTF = mybir.ActivationFunctionType

NB = 32
SR = 1.0 / 2.355
PT = 1.0 / (2.0 * SR * SR)          # normalized preterm = 2.355^2/2
SQPT = float(np.sqrt(PT))
EPS = float(np.finfo(np.float32).eps)
NCORES = 8
N = 128 * 128 * 128                  # 2097152 voxels
V = N // NCORES                      # 262144 per core
P = 128
F = V // P                           # 2048 free-dim per partition

# ---------------------------------------------------------------- ladders
Z99 = 2.3263478740408408             # N(0,1) 0.99 quantile (inputs are randn)
FINE_RUNGS = 3                       # per quantile
FINE_DELTA = 0.008
COARSE = [-4.0, 4.0]                 # fallback bracket rungs
NLAD = 2 * FINE_RUNGS + len(COARSE)  # 8 per tensor


def _default_ladder():
    lo = [-Z99 + FINE_DELTA * (j - FINE_RUNGS / 2 + 0.5) for j in range(FINE_RUNGS)]
    hi = [Z99 + FINE_DELTA * (j - FINE_RUNGS / 2 + 0.5) for j in range(FINE_RUNGS)]
    return lo + hi + COARSE


def _build_phase1():
    """Counts of (value >= t_j) for 2*NLAD runtime thresholds.

    cst layout [128, 4*NLAD]: cols [0,2N): thresholds (yt then yp),
    cols [2N,4N): negated thresholds (Sign biases). Rungs alternate
    between DVE (is_ge+accum) and ACT (Sign+accum), both ~2.2us/op."""
    nc = bacc.Bacc("TRN2", target_bir_lowering=False, debug=False,
                   num_devices=NCORES)
    yt_d = nc.dram_tensor("yt", [P, F], F32, kind="ExternalInput").ap()
    yp_d = nc.dram_tensor("yp", [P, F], F32, kind="ExternalInput").ap()
    cst_d = nc.dram_tensor("cst", [P, 4 * NLAD], F32, kind="ExternalInput").ap()
    ncols = 2 * NLAD
    out_d = nc.dram_tensor("cnt", [P, ncols], F32, kind="ExternalOutput").ap()

    with tile.TileContext(nc) as tc:
        with (
            tc.tile_pool(name="io", bufs=1) as io_pool,
            tc.tile_pool(name="scr", bufs=4) as scr_pool,
            tc.tile_pool(name="stat", bufs=1) as stat_pool,
        ):
            yt = io_pool.tile([P, F], F32)
            yp = io_pool.tile([P, F], F32)
            cst = io_pool.tile([P, 4 * NLAD], F32)
            nc.sync.dma_start(cst[:], cst_d)
            nc.sync.dma_start(yt[:], yt_d)
            nc.sync.dma_start(yp[:], yp_d)

            cnt = stat_pool.tile([P, ncols], F32)      # per-partition counts

            # Even cols -> DVE is_ge, odd cols -> ACT Sign.
            for col in range(ncols):
                src = yt if col < NLAD else yp
                if col % 2 == 0:
                    scr = scr_pool.tile([P, F], BF16, tag="scr")
                    nc.vector.tensor_scalar(
                        out=scr[:], in0=src[:], scalar1=cst[:, col:col + 1],
                        scalar2=None, op0=ALU.is_ge, op1=ALU.add,
                        accum_out=cnt[:, col:col + 1])
                else:
                    scr = scr_pool.tile([P, F], BF16, tag="ascr")
                    nc.scalar.activation(
                        scr[:], src[:], ACTF.Sign,
                        bias=cst[:, ncols + col:ncols + col + 1],
                        scale=1.0, accum_out=cnt[:, col:col + 1])

            # Host folds partitions; just DMA the per-partition counts out.
            nc.sync.dma_start(out_d, cnt[:])
    nc.compile()
    return nc


def _build_phase2():
    """Main pass. Direction A: target=y_pred, pred=y_true (mirrors
    correlation_ratio(y_true, y_pred)); direction B swaps.

    Only two distinct clipped tensors exist: ytc=clip(yt, qyt) and
    ypc=clip(yp, qyp); dir A uses (target=ypc, x=ytc), dir B swaps.

    cst layout [128, 72]:
      [0,32)  DErf bias dir A: -SQPT*(ivf_A*f_min_A + k + 0.5)
      [32,64) DErf bias dir B
      64: scale dir A = SQPT*ivf_A;  65: scale dir B
      66,67: yt_lo, yt_hi;  68,69: yp_lo, yp_hi."""
    nc = bacc.Bacc("TRN2", target_bir_lowering=False, debug=False,
                   num_devices=NCORES)
    yt_d = nc.dram_tensor("yt", [P, F], F32, kind="ExternalInput").ap()
    yp_d = nc.dram_tensor("yp", [P, F], F32, kind="ExternalInput").ap()
    cst_d = nc.dram_tensor("cst", [P, 72], F32, kind="ExternalInput").ap()
    # per direction: 32 S0 | 32 S1 | sumx | sumx2 -> 66 cols; A then B
    out_d = nc.dram_tensor("stats", [P, 132], F32, kind="ExternalOutput").ap()

    with tile.TileContext(nc) as tc:
        with (
            tc.tile_pool(name="io", bufs=1) as io_pool,
            tc.tile_pool(name="norm", bufs=1) as norm_pool,
            tc.tile_pool(name="w", bufs=4) as w_pool,
            tc.tile_pool(name="scr", bufs=3) as scr_pool,
            tc.tile_pool(name="stat", bufs=1) as stat_pool,
        ):
            yt = io_pool.tile([P, F], F32)
            yp = io_pool.tile([P, F], F32)
            cst = io_pool.tile([P, 72], F32)
            nc.sync.dma_start(cst[:], cst_d)
            nc.sync.dma_start(yp[:], yp_d)
            nc.sync.dma_start(yt[:], yt_d)

            stats = stat_pool.tile([P, 132], F32)

            # --- clipped tensors (f32 for ACT input, bf16 for the x role)
            clipped = {}
            for name, src, lo, hi in (
                ("yp", yp, cst[:, 68:69], cst[:, 69:70]),
                ("yt", yt, cst[:, 66:67], cst[:, 67:68]),
            ):
                cf = norm_pool.tile([P, F], F32, tag=f"{name}cf")
                nc.vector.tensor_scalar(
                    out=cf[:], in0=src[:], scalar1=hi,
                    scalar2=lo, op0=ALU.min, op1=ALU.max)
                cb = norm_pool.tile([P, F], BF16, tag=f"{name}cb")
                nc.vector.tensor_scalar(
                    out=cb[:], in0=cf[:], scalar1=1.0,
                    scalar2=None, op0=ALU.mult)
                clipped[name] = (cf, cb)

            # --- SX / SX2 per direction (x role: dirA->ytc, dirB->ypc)
            # ACT Copy+accum and Square+accum; batched before any DErf so
            # the activation table set switches only once.
            for di, xname in ((0, "yt"), (1, "yp")):
                base = 66 * di
                xb = clipped[xname][1]
                xcp = scr_pool.tile([P, F], BF16, tag="xcp")
                nc.scalar.activation(xcp[:], xb[:], ACTF.Copy,
                                     bias=0.0, scale=1.0,
                                     accum_out=stats[:, base + 64:base + 65])
                xsq = scr_pool.tile([P, F], BF16, tag="xsq")
                nc.scalar.activation(xsq[:], xb[:], ACTF.Square,
                                     bias=0.0, scale=1.0,
                                     accum_out=stats[:, base + 65:base + 66])

            # --- per (direction, bin): ACT DErf (S0) + DVE w*x (S1)
            for di, (tname, xname) in ((0, ("yp", "yt")), (1, ("yt", "yp"))):
                base = 66 * di
                tc_f32 = clipped[tname][0]
                xb = clipped[xname][1]
                scale = cst[:, 64 + di:65 + di]
                for k in range(NB):
                    w = w_pool.tile([P, F], BF16, tag="w")
                    nc.scalar.activation(
                        w[:], tc_f32[:], ACTF.Derivative_Erf,
                        bias=cst[:, 32 * di + k:32 * di + k + 1], scale=scale,
                        accum_out=stats[:, base + k:base + k + 1])
                    wx = scr_pool.tile([P, F], BF16, tag="wx")
                    nc.vector.scalar_tensor_tensor(
                        out=wx[:], in0=w[:], scalar=1.0, in1=xb[:],
                        op0=ALU.mult, op1=ALU.mult,
                        accum_out=stats[:, base + 32 + k:base + 32 + k + 1])

            # Host folds partitions; DMA the whole per-partition stats tile.
            nc.sync.dma_start(out_d, stats[:])
    nc.compile()
    return nc


_NC_CACHE = {}


def _get_nc(which):
    if which not in _NC_CACHE:
        _NC_CACHE[which] = _build_phase1() if which == "p1" else _build_phase2()
    return _NC_CACHE[which]


def _run(nc, in_maps, trace=False):
    return bass_utils.run_bass_kernel_spmd(
        nc, in_maps, core_ids=list(range(NCORES)), trace=trace)


def _p1_cst(ladder_yt, ladder_yp):
    thr = np.array(list(ladder_yt) + list(ladder_yp), dtype=np.float32)
    cst = np.concatenate([thr, -thr]).reshape(1, -1)
    return np.ascontiguousarray(np.broadcast_to(cst, (P, 4 * NLAD)), dtype=np.float32)


def _p2_cst(qyt_lo, qyt_hi, qyp_lo, qyp_hi):
    row = np.zeros(72, dtype=np.float32)
    ks = np.arange(NB, dtype=np.float64)
    for di, ((tlo, thi), _) in enumerate(
        (((qyp_lo, qyp_hi), None), ((qyt_lo, qyt_hi), None))):
        tlo32 = np.float32(tlo); thi32 = np.float32(thi)
        fbs = np.float32((thi32 - tlo32) / NB)
        ivf = np.float64(np.float32(1.0) / fbs)
        row[32 * di:32 * di + NB] = (-SQPT * (ivf * tlo32 + ks + 0.5)
                                     ).astype(np.float32)
        row[64 + di] = np.float32(SQPT * ivf)
    row[66] = np.float32(qyt_lo); row[67] = np.float32(qyt_hi)
    row[68] = np.float32(qyp_lo); row[69] = np.float32(qyp_hi)
    return np.ascontiguousarray(np.broadcast_to(row.reshape(1, -1), (P, 72)),
                                dtype=np.float32)


def _interp_quantile(thresholds, counts_ge, pos):
    """CDF interpolation: counts_ge[i] = #(values >= t_i) globally.
    pos = q*(N-1) fractional order-statistic position (ascending)."""
    below = N - np.asarray(counts_ge, dtype=np.float64)   # count(< t_i)
    r = pos + 1.0
    best = None
    for i in range(len(thresholds) - 1):
        if thresholds[i + 1] <= thresholds[i]:
            continue
        if below[i] <= r <= below[i + 1] and below[i + 1] > below[i]:
            frac = (r - below[i]) / (below[i + 1] - below[i])
            est = thresholds[i] + frac * (thresholds[i + 1] - thresholds[i])
            width = thresholds[i + 1] - thresholds[i]
            if best is None or width < best[0]:
                best = (width, est)
    return None if best is None else best[1]


def _quantiles_from_counts(ladder, counts_ge):
    nf = FINE_RUNGS
    q01 = _interp_quantile(ladder[:nf], counts_ge[:nf], 0.01 * (N - 1))
    q99 = _interp_quantile(ladder[nf:2 * nf], counts_ge[nf:2 * nf], 0.99 * (N - 1))
    co_thr, co_cnt = ladder[2 * nf:], counts_ge[2 * nf:]
    ok = (q01 is not None, q99 is not None)
    if q01 is None:
        q01 = _interp_quantile(co_thr, co_cnt, 0.01 * (N - 1))
    if q99 is None:
        q99 = _interp_quantile(co_thr, co_cnt, 0.99 * (N - 1))
    return (q01, ok[0]), (q99, ok[1])


def _counts_from_phase1(res_cnt):
    arr = np.stack([np.asarray(r, dtype=np.float64).reshape(P, -1).sum(axis=0)
                    for r in res_cnt])
    tot = arr.sum(axis=0)
    counts = np.empty(2 * NLAD)
    for col in range(2 * NLAD):
        if col % 2 == 0:
            counts[col] = tot[col]                      # is_ge count
        else:
            counts[col] = 0.5 * (tot[col] + N)          # sign sum -> count>=
    return counts


def _final_algebra(stats_sum):
    out = 0.0
    for di in range(2):
        base = 66 * di
        S0 = stats_sum[base:base + 32]
        S1 = stats_sum[base + 32:base + 64]
        SX = stats_sum[base + 64]
        SX2 = stats_sum[base + 65]
        # S0/S1 carry the DErf 2/sqrt(pi) factor; it cancels in mi and bgv.
        tm = SX / N
        mi = S1 / (S0 + EPS)
        bgv = float((S0 * (mi - tm) ** 2).sum() / (S0.sum() + EPS))
        tv = (SX2 - N * tm * tm) / (N - 1)
        out += bgv / (tv + EPS)
    return -out / 2.0


def kernel(y_true, y_pred):
    yt = np.ascontiguousarray(np.asarray(y_true, dtype=np.float32).reshape(-1))
    yp = np.ascontiguousarray(np.asarray(y_pred, dtype=np.float32).reshape(-1))
    assert yt.size == N and yp.size == N
    yt_sh = yt.reshape(NCORES, P, F)
    yp_sh = yp.reshape(NCORES, P, F)

    # ---- phase 1: quantiles
    ladder = _default_ladder()
    ladder_yt = ladder_yp = ladder
    nc1 = _get_nc("p1")
    for _attempt in range(4):
        cst1 = _p1_cst(ladder_yt, ladder_yp)
        in_maps = [{"yt": yt_sh[c], "yp": yp_sh[c], "cst": cst1}
                   for c in range(NCORES)]
        r1 = _run(nc1, in_maps)
        counts = _counts_from_phase1([r["cnt"] for r in r1.results])
        (qyt_lo, ok1), (qyt_hi, ok2) = _quantiles_from_counts(ladder_yt, counts[:NLAD])
        (qyp_lo, ok3), (qyp_hi, ok4) = _quantiles_from_counts(ladder_yp, counts[NLAD:])
        if ok1 and ok2 and ok3 and ok4:
            break
        # Fine ladders missed (data not ~randn): refine around the coarse
        # bracket using the same compiled NEFF. If even the coarse bracket
        # missed (quantile is None), widen it geometrically.
        span = max(abs(c) for c in COARSE)
        wide_lo, wide_hi = -span * 8, span * 8

        def lad(lo_c, hi_c):
            if lo_c is None or hi_c is None:
                return ([wide_lo + (wide_hi - wide_lo) * j / (2 * FINE_RUNGS - 1)
                         for j in range(2 * FINE_RUNGS)] + [wide_lo * 4, wide_hi * 4])
            d = max((hi_c - lo_c) / 256.0, 1e-7)
            return ([lo_c + d * (j - FINE_RUNGS / 2 + 0.5) for j in range(FINE_RUNGS)]
                    + [hi_c + d * (j - FINE_RUNGS / 2 + 0.5) for j in range(FINE_RUNGS)]
                    + [lo_c - 32 * d, hi_c + 32 * d])
        ladder_yt = lad(qyt_lo, qyt_hi)
        ladder_yp = lad(qyp_lo, qyp_hi)

    # ---- phase 2: main pass
    cst2 = _p2_cst(qyt_lo, qyt_hi, qyp_lo, qyp_hi)
    nc2 = _get_nc("p2")
    in_maps2 = [{"yt": yt_sh[c], "yp": yp_sh[c], "cst": cst2}
                for c in range(NCORES)]
    r2 = _run(nc2, in_maps2)
    stats = np.stack([np.asarray(r["stats"], dtype=np.float64
                                 ).reshape(P, 132).sum(axis=0)
                      for r in r2.results]).sum(axis=0)
    return np.array(_final_algebra(stats), dtype=np.float32)


# revision 5
# speedup vs baseline: 2.1196x; 1.0239x over previous
"""CorrRatio (symmetric correlation-ratio loss) on 8 Trainium2 NeuronCores.

Strategy
--------
Input: y_true, y_pred f32 (1,1,128,128,128) -> N = 2^21 voxels, sharded
contiguously across 8 cores as [128, 2048] f32 tiles (all reductions are
order-independent, so contiguous sharding is exact).

Phase 1 (slim NEFF): exact threshold-count ladders around the expected
0.01/0.99 quantile locations (randn inputs) + 2 coarse safety rungs per
tensor. Counts split between ACT (Sign+accum) and DVE (is_ge+accum) so
both engines run in parallel; per-partition counts are folded on host
(the "small all-reduce"). Thresholds are runtime inputs; a miss
(non-randn data) re-runs the same NEFF with a refined ladder.

Phase 2 (main NEFF): per tensor one DVE clip op (f32) + one clipped bf16
cast. Per (direction, bin): ONE ACT op
  Derivative_Erf(s*tc + b_k) = (2/sqrt(pi)) * exp(-PT*(yn - k - .5)^2)
(the normalize affine yn=(tc-f_min)*inv_fbs is folded into scale/bias;
bf16 out) with accum_out -> S0[k], and ONE DVE scalar_tensor_tensor
w*x (bf16) with accum_out -> S1[k]. Measured facts driving this: ACT is
~2.0us/2048 elems dtype-independent; ANY accumulating DVE op is 1x mode
(~2.2us) — so 1 ACT + 1 DVE op per bin saturates both engines evenly.
Host folds partition partials and does the final algebra in f64 (the
DErf 2/sqrt(pi) scale cancels in the ratios).
"""

import numpy as np

import concourse.bacc as bacc
import concourse.bass as bass
import concourse.mybir as mybir
import concourse.tile as tile
from concourse import bass_utils

F32 = mybir.dt.float32
BF16 = mybir.dt.bfloat16
ALU = mybir.AluOpType
ACTF = mybir.ActivationFunctionType

NB = 32
SR = 1.0 / 2.355
PT = 1.0 / (2.0 * SR * SR)          # normalized preterm = 2.355^2/2
SQPT = float(np.sqrt(PT))
EPS = float(np.finfo(np.float32).eps)
NCORES = 8
N = 128 * 128 * 128                  # 2097152 voxels
V = N // NCORES                      # 262144 per core
P = 128
F = V // P                           # 2048 free-dim per partition

# ---------------------------------------------------------------- ladders
Z99 = 2.3263478740408408             # N(0,1) 0.99 quantile (inputs are randn)
FINE_RUNGS = 3                       # per quantile
FINE_DELTA = 0.004
COARSE = [-4.0, 4.0]                 # fallback bracket rungs
NLAD = 2 * FINE_RUNGS + len(COARSE)  # 8 per tensor


def _default_ladder():
    lo = [-Z99 + FINE_DELTA * (j - FINE_RUNGS / 2 + 0.5) for j in range(FINE_RUNGS)]
    hi = [Z99 + FINE_DELTA * (j - FINE_RUNGS / 2 + 0.5) for j in range(FINE_RUNGS)]
    return lo + hi + COARSE


def _build_phase1():
    """Counts of (value >= t_j) for 2*NLAD runtime thresholds.

    cst layout [128, 4*NLAD]: cols [0,2N): thresholds (yt then yp),
    cols [2N,4N): negated thresholds (Sign biases). Rungs alternate
    between DVE (is_ge+accum) and ACT (Sign+accum), both ~2.2us/op."""
    nc = bacc.Bacc("TRN2", target_bir_lowering=False, debug=False,
                   num_devices=NCORES)
    yt_d = nc.dram_tensor("yt", [P, F], F32, kind="ExternalInput").ap()
    yp_d = nc.dram_tensor("yp", [P, F], F32, kind="ExternalInput").ap()
    cst_d = nc.dram_tensor("cst", [P, 4 * NLAD], F32, kind="ExternalInput").ap()
    ncols = 2 * NLAD
    out_d = nc.dram_tensor("cnt", [P, ncols], F32, kind="ExternalOutput").ap()

    with tile.TileContext(nc) as tc:
        with (
            tc.tile_pool(name="io", bufs=1) as io_pool,
            tc.tile_pool(name="scr", bufs=4) as scr_pool,
            tc.tile_pool(name="stat", bufs=1) as stat_pool,
        ):
            yt = io_pool.tile([P, F], F32)
            yp = io_pool.tile([P, F], F32)
            cst = io_pool.tile([P, 4 * NLAD], F32)
            nc.sync.dma_start(cst[:], cst_d)
            nc.sync.dma_start(yt[:], yt_d)
            nc.sync.dma_start(yp[:], yp_d)

            cnt = stat_pool.tile([P, ncols], F32)      # per-partition counts

            # Even cols -> DVE is_ge, odd cols -> ACT Sign.
            for col in range(ncols):
                src = yt if col < NLAD else yp
                if col % 2 == 0:
                    scr = scr_pool.tile([P, F], BF16, tag="scr")
                    nc.vector.tensor_scalar(
                        out=scr[:], in0=src[:], scalar1=cst[:, col:col + 1],
                        scalar2=None, op0=ALU.is_ge, op1=ALU.add,
                        accum_out=cnt[:, col:col + 1])
                else:
                    scr = scr_pool.tile([P, F], BF16, tag="ascr")
                    nc.scalar.activation(
                        scr[:], src[:], ACTF.Sign,
                        bias=cst[:, ncols + col:ncols + col + 1],
                        scale=1.0, accum_out=cnt[:, col:col + 1])

            # Host folds partitions; just DMA the per-partition counts out.
            nc.sync.dma_start(out_d, cnt[:])
    nc.compile()
    return nc


def _build_phase2():
    """Main pass. Direction A: target=y_pred, pred=y_true (mirrors
    correlation_ratio(y_true, y_pred)); direction B swaps.

    Only two distinct clipped tensors exist: ytc=clip(yt, qyt) and
    ypc=clip(yp, qyp); dir A uses (target=ypc, x=ytc), dir B swaps.

    cst layout [128, 72]:
      [0,32)  DErf bias dir A: -SQPT*(ivf_A*f_min_A + k + 0.5)
      [32,64) DErf bias dir B
      64: scale dir A = SQPT*ivf_A;  65: scale dir B
      66,67: yt_lo, yt_hi;  68,69: yp_lo, yp_hi."""
    nc = bacc.Bacc("TRN2", target_bir_lowering=False, debug=False,
                   num_devices=NCORES)
    yt_d = nc.dram_tensor("yt", [P, F], F32, kind="ExternalInput").ap()
    yp_d = nc.dram_tensor("yp", [P, F], F32, kind="ExternalInput").ap()
    cst_d = nc.dram_tensor("cst", [P, 72], F32, kind="ExternalInput").ap()
    # per direction: 32 S0 | 32 S1 | sumx | sumx2 -> 66 cols; A then B
    out_d = nc.dram_tensor("stats", [P, 132], F32, kind="ExternalOutput").ap()

    with tile.TileContext(nc) as tc:
        with (
            tc.tile_pool(name="io", bufs=1) as io_pool,
            tc.tile_pool(name="norm", bufs=1) as norm_pool,
            tc.tile_pool(name="w", bufs=4) as w_pool,
            tc.tile_pool(name="scr", bufs=3) as scr_pool,
            tc.tile_pool(name="stat", bufs=1) as stat_pool,
        ):
            yt = io_pool.tile([P, F], F32)
            yp = io_pool.tile([P, F], F32)
            cst = io_pool.tile([P, 72], F32)
            nc.sync.dma_start(cst[:], cst_d)
            nc.sync.dma_start(yp[:], yp_d)
            nc.sync.dma_start(yt[:], yt_d)

            stats = stat_pool.tile([P, 132], F32)

            # --- clipped tensors (f32 for ACT input, bf16 for the x role)
            # Order: clip yp (dir A DErf input), clip yt, cast yt (dir A wx
            # input), cast yp — gets both the first DErf and first wx
            # running as early as possible.
            ypcf = norm_pool.tile([P, F], F32, tag="ypcf")
            nc.vector.tensor_scalar(
                out=ypcf[:], in0=yp[:], scalar1=cst[:, 69:70],
                scalar2=cst[:, 68:69], op0=ALU.min, op1=ALU.max)
            ytcf = norm_pool.tile([P, F], F32, tag="ytcf")
            nc.vector.tensor_scalar(
                out=ytcf[:], in0=yt[:], scalar1=cst[:, 67:68],
                scalar2=cst[:, 66:67], op0=ALU.min, op1=ALU.max)
            ytcb = norm_pool.tile([P, F], BF16, tag="ytcb")
            nc.vector.tensor_scalar(
                out=ytcb[:], in0=ytcf[:], scalar1=1.0,
                scalar2=None, op0=ALU.mult)
            ypcb = norm_pool.tile([P, F], BF16, tag="ypcb")
            nc.vector.tensor_scalar(
                out=ypcb[:], in0=ypcf[:], scalar1=1.0,
                scalar2=None, op0=ALU.mult)
            clipped = {"yp": (ypcf, ypcb), "yt": (ytcf, ytcb)}

            # --- per (direction, bin): ACT DErf (S0) + DVE w*x (S1)
            for di, (tname, xname) in ((0, ("yp", "yt")), (1, ("yt", "yp"))):
                base = 66 * di
                tc_f32 = clipped[tname][0]
                xb = clipped[xname][1]
                scale = cst[:, 64 + di:65 + di]
                for k in range(NB):
                    w = w_pool.tile([P, F], BF16, tag="w")
                    nc.scalar.activation(
                        w[:], tc_f32[:], ACTF.Derivative_Erf,
                        bias=cst[:, 32 * di + k:32 * di + k + 1], scale=scale,
                        accum_out=stats[:, base + k:base + k + 1])
                    wx = scr_pool.tile([P, F], BF16, tag="wx")
                    nc.vector.scalar_tensor_tensor(
                        out=wx[:], in0=w[:], scalar=1.0, in1=xb[:],
                        op0=ALU.mult, op1=ALU.mult,
                        accum_out=stats[:, base + 32 + k:base + 32 + k + 1])

            # --- SX / SX2 per direction (x role: dirA->ytc, dirB->ypc).
            # Emitted AFTER the bin work so these ACT ops fill ACT's tail
            # instead of starving DVE of w tiles early on.
            for di, xname in ((0, "yt"), (1, "yp")):
                base = 66 * di
                xb = clipped[xname][1]
                xcp = scr_pool.tile([P, F], BF16, tag="xcp")
                nc.scalar.activation(xcp[:], xb[:], ACTF.Copy,
                                     bias=0.0, scale=1.0,
                                     accum_out=stats[:, base + 64:base + 65])
                xsq = scr_pool.tile([P, F], BF16, tag="xsq")
                nc.scalar.activation(xsq[:], xb[:], ACTF.Square,
                                     bias=0.0, scale=1.0,
                                     accum_out=stats[:, base + 65:base + 66])

            # Host folds partitions; DMA the whole per-partition stats tile.
            nc.sync.dma_start(out_d, stats[:])
    nc.compile()
    return nc


_NC_CACHE = {}


def _get_nc(which):
    if which not in _NC_CACHE:
        _NC_CACHE[which] = _build_phase1() if which == "p1" else _build_phase2()
    return _NC_CACHE[which]


def _run(nc, in_maps, trace=False):
    return bass_utils.run_bass_kernel_spmd(
        nc, in_maps, core_ids=list(range(NCORES)), trace=trace)


def _p1_cst(ladder_yt, ladder_yp):
    thr = np.array(list(ladder_yt) + list(ladder_yp), dtype=np.float32)
    cst = np.concatenate([thr, -thr]).reshape(1, -1)
    return np.ascontiguousarray(np.broadcast_to(cst, (P, 4 * NLAD)), dtype=np.float32)


def _p2_cst(qyt_lo, qyt_hi, qyp_lo, qyp_hi):
    row = np.zeros(72, dtype=np.float32)
    ks = np.arange(NB, dtype=np.float64)
    for di, ((tlo, thi), _) in enumerate(
        (((qyp_lo, qyp_hi), None), ((qyt_lo, qyt_hi), None))):
        tlo32 = np.float32(tlo); thi32 = np.float32(thi)
        fbs = np.float32((thi32 - tlo32) / NB)
        ivf = np.float64(np.float32(1.0) / fbs)
        row[32 * di:32 * di + NB] = (-SQPT * (ivf * tlo32 + ks + 0.5)
                                     ).astype(np.float32)
        row[64 + di] = np.float32(SQPT * ivf)
    row[66] = np.float32(qyt_lo); row[67] = np.float32(qyt_hi)
    row[68] = np.float32(qyp_lo); row[69] = np.float32(qyp_hi)
    return np.ascontiguousarray(np.broadcast_to(row.reshape(1, -1), (P, 72)),
                                dtype=np.float32)


def _interp_quantile(thresholds, counts_ge, pos):
    """CDF interpolation: counts_ge[i] = #(values >= t_i) globally.
    pos = q*(N-1) fractional order-statistic position (ascending)."""
    below = N - np.asarray(counts_ge, dtype=np.float64)   # count(< t_i)
    r = pos + 1.0
    best = None
    for i in range(len(thresholds) - 1):
        if thresholds[i + 1] <= thresholds[i]:
            continue
        if below[i] <= r <= below[i + 1] and below[i + 1] > below[i]:
            frac = (r - below[i]) / (below[i + 1] - below[i])
            est = thresholds[i] + frac * (thresholds[i + 1] - thresholds[i])
            width = thresholds[i + 1] - thresholds[i]
            if best is None or width < best[0]:
                best = (width, est)
    return None if best is None else best[1]


def _quantiles_from_counts(ladder, counts_ge):
    nf = FINE_RUNGS
    q01 = _interp_quantile(ladder[:nf], counts_ge[:nf], 0.01 * (N - 1))
    q99 = _interp_quantile(ladder[nf:2 * nf], counts_ge[nf:2 * nf], 0.99 * (N - 1))
    co_thr, co_cnt = ladder[2 * nf:], counts_ge[2 * nf:]
    ok = (q01 is not None, q99 is not None)
    if q01 is None:
        q01 = _interp_quantile(co_thr, co_cnt, 0.01 * (N - 1))
    if q99 is None:
        q99 = _interp_quantile(co_thr, co_cnt, 0.99 * (N - 1))
    return (q01, ok[0]), (q99, ok[1])


def _counts_from_phase1(res_cnt):
    arr = np.stack([np.asarray(r, dtype=np.float64).reshape(P, -1).sum(axis=0)
                    for r in res_cnt])
    tot = arr.sum(axis=0)
    counts = np.empty(2 * NLAD)
    for col in range(2 * NLAD):
        if col % 2 == 0:
            counts[col] = tot[col]                      # is_ge count
        else:
            counts[col] = 0.5 * (tot[col] + N)          # sign sum -> count>=
    return counts


def _final_algebra(stats_sum):
    out = 0.0
    for di in range(2):
        base = 66 * di
        S0 = stats_sum[base:base + 32]
        S1 = stats_sum[base + 32:base + 64]
        SX = stats_sum[base + 64]
        SX2 = stats_sum[base + 65]
        # S0/S1 carry the DErf 2/sqrt(pi) factor; it cancels in mi and bgv.
        tm = SX / N
        mi = S1 / (S0 + EPS)
        bgv = float((S0 * (mi - tm) ** 2).sum() / (S0.sum() + EPS))
        tv = (SX2 - N * tm * tm) / (N - 1)
        out += bgv / (tv + EPS)
    return -out / 2.0


def kernel(y_true, y_pred):
    yt = np.ascontiguousarray(np.asarray(y_true, dtype=np.float32).reshape(-1))
    yp = np.ascontiguousarray(np.asarray(y_pred, dtype=np.float32).reshape(-1))
    assert yt.size == N and yp.size == N
    yt_sh = yt.reshape(NCORES, P, F)
    yp_sh = yp.reshape(NCORES, P, F)

    # ---- phase 1: quantiles
    ladder = _default_ladder()
    ladder_yt = ladder_yp = ladder
    nc1 = _get_nc("p1")
    for _attempt in range(4):
        cst1 = _p1_cst(ladder_yt, ladder_yp)
        in_maps = [{"yt": yt_sh[c], "yp": yp_sh[c], "cst": cst1}
                   for c in range(NCORES)]
        r1 = _run(nc1, in_maps)
        counts = _counts_from_phase1([r["cnt"] for r in r1.results])
        (qyt_lo, ok1), (qyt_hi, ok2) = _quantiles_from_counts(ladder_yt, counts[:NLAD])
        (qyp_lo, ok3), (qyp_hi, ok4) = _quantiles_from_counts(ladder_yp, counts[NLAD:])
        if ok1 and ok2 and ok3 and ok4:
            break
        # Fine ladders missed (data not ~randn): refine around the coarse
        # bracket using the same compiled NEFF. If even the coarse bracket
        # missed (quantile is None), widen it geometrically.
        span = max(abs(c) for c in COARSE)
        wide_lo, wide_hi = -span * 8, span * 8

        def lad(lo_c, hi_c):
            if lo_c is None or hi_c is None:
                return ([wide_lo + (wide_hi - wide_lo) * j / (2 * FINE_RUNGS - 1)
                         for j in range(2 * FINE_RUNGS)] + [wide_lo * 4, wide_hi * 4])
            d = max((hi_c - lo_c) / 256.0, 1e-7)
            return ([lo_c + d * (j - FINE_RUNGS / 2 + 0.5) for j in range(FINE_RUNGS)]
                    + [hi_c + d * (j - FINE_RUNGS / 2 + 0.5) for j in range(FINE_RUNGS)]
                    + [lo_c - 32 * d, hi_c + 32 * d])
        ladder_yt = lad(qyt_lo, qyt_hi)
        ladder_yp = lad(qyp_lo, qyp_hi)

    # ---- phase 2: main pass
    cst2 = _p2_cst(qyt_lo, qyt_hi, qyp_lo, qyp_hi)
    nc2 = _get_nc("p2")
    in_maps2 = [{"yt": yt_sh[c], "yp": yp_sh[c], "cst": cst2}
                for c in range(NCORES)]
    r2 = _run(nc2, in_maps2)
    stats = np.stack([np.asarray(r["stats"], dtype=np.float64
                                 ).reshape(P, 132).sum(axis=0)
                      for r in r2.results]).sum(axis=0)
    return np.array(_final_algebra(stats), dtype=np.float32)
